# revision 1
# baseline (speedup 1.0000x reference)
"""GCN (3x GCNConv + linear + log_softmax) on 8 Trainium2 NeuronCores.

Formulation: gcn_conv(h, W) = dinv * ((A + I) @ u) + b   with u = dinv * (h @ W)
so the per-edge norm folds into row scalings and message passing is a pure
gather + segmented sum.

Sharding: nodes are sharded across the 8 cores (each core owns its dst rows).
Per layer each core computes its u-slice, an AllGather replicates the full u
table to every core's DRAM, then each core gathers its in-edge messages with
dma_gather (512B rows, full DMA line rate) and scatter-adds them with one-hot
PE matmuls accumulating in PSUM per 128-dst window.

All graph preprocessing (edge partitioning, dst->window packing, index
wrapping) happens on the host in numpy. One SPMD program is shared by all 8
cores; everything data-dependent per core is an input tensor.
"""

import sys

sys.path.insert(0, "/opt/trn_rl_repo")

import numpy as np
from concourse import bass, bacc, tile, mybir
from concourse.bass_utils import run_bass_kernel_spmd

M = 8          # cores
P = 128        # partitions
CH = 16        # gather chunk size in message tiles (CH*128 idxs per dma_gather)
A_CORES = 5    # cores 0..4 -> table A, cores 5..7 -> table B (int16 idx limit)

# debug bisect knobs
USE_COLLECTIVE = True
USE_GATHER = True
USE_MM = True
USE_UCOMPUTE = True

F32 = mybir.dt.float32
I16 = mybir.dt.int16


# ----------------------------------------------------------------------------
# Host-side schedule construction
# ----------------------------------------------------------------------------

def _pack_core(d_loc, degA, degB, npc, W, capA, capB):
    """Assign each local dst node to a (window, slot). Returns assignment
    [npc] -> window, or None if infeasible."""
    order = np.argsort(-(degA + degB), kind="stable")
    remA = np.full(W, capA, np.int64)
    remB = np.full(W, capB, np.int64)
    rem_slots = np.full(W, P, np.int64)
    win_of = np.full(npc, -1, np.int64)
    for d in order:
        a, b = degA[d], degB[d]
        # best fit: feasible window with max remaining total capacity
        feas = (rem_slots > 0) & (remA >= a) & (remB >= b)
        if not feas.any():
            return None
        score = np.where(feas, remA + remB, -1)
        w = int(np.argmax(score))
        win_of[d] = w
        remA[w] -= a
        remB[w] -= b
        rem_slots[w] -= 1
    return win_of


def build_schedule(edge_index, n_nodes):
    N = n_nodes
    E = edge_index.shape[1]
    npc = N // M
    assert npc * M == N

    src = np.asarray(edge_index[0], dtype=np.int64)
    dst = np.asarray(edge_index[1], dtype=np.int64)
    deg = np.bincount(dst, minlength=N).astype(np.float64) + 1.0
    dinv = (1.0 / np.sqrt(deg)).astype(np.float32)

    src_owner = src // npc
    is_A = src_owner < A_CORES

    # per-core edge sets and per-dst A/B degrees
    core_of_dst = dst // npc
    edge_core = core_of_dst
    degA_all = np.zeros((M, npc), np.int64)
    degB_all = np.zeros((M, npc), np.int64)
    for c in range(M):
        sel = edge_core == c
        dl = dst[sel] - c * npc
        degA_all[c] = np.bincount(dl[is_A[sel]], minlength=npc)
        degB_all[c] = np.bincount(dl[~is_A[sel]], minlength=npc)

    # choose uniform (W, TA, TB)
    W = max((npc + P - 1) // P, 1)
    maxA = max(int(degA_all[c].sum()) for c in range(M))
    maxB = max(int(degB_all[c].sum()) for c in range(M))
    TA = max((maxA // W + P - 1) // P + 1, 1)
    TB = max((maxB // W + P - 1) // P + 1, 1)

    for _ in range(64):
        capA, capB = TA * P, TB * P
        wins = []
        ok = True
        for c in range(M):
            w = _pack_core(None, degA_all[c], degB_all[c], npc, W, capA, capB)
            if w is None:
                ok = False
                break
            wins.append(w)
        if ok:
            break
        # grow the tighter capacity
        slackA = min(W * capA - int(degA_all[c].sum()) for c in range(M))
        slackB = min(W * capB - int(degB_all[c].sum()) for c in range(M))
        if slackA <= slackB:
            TA += 1
        else:
            TB += 1
    else:
        raise RuntimeError("packing failed")

    NPS = W * P          # padded rows per core
    A_ROWS = A_CORES * NPS
    B_ROWS = (M - A_CORES) * NPS
    assert A_ROWS <= 32768 and B_ROWS <= 32768, (A_ROWS, B_ROWS)

    # slot assignment within windows + global new ids
    newid = np.full(N, -1, np.int64)
    slot_orig = np.full((M, NPS), -1, np.int64)  # (c, w*128+j) -> orig node
    for c in range(M):
        win_of = wins[c]
        next_slot = np.zeros(W, np.int64)
        for d in range(npc):
            w = win_of[d]
            j = next_slot[w]
            next_slot[w] += 1
            newid[c * npc + d] = c * NPS + w * P + j
            slot_orig[c, w * P + j] = c * npc + d
        assert (next_slot <= P).all()

    T = TA + TB
    LA, LB = W * TA * P, W * TB * P

    per_core = []
    for c in range(M):
        sel = edge_core == c
        s_c = src[sel]
        d_c = dst[sel] - c * npc
        a_c = is_A[sel]
        win_of = wins[c]

        idxA = np.zeros(LA, np.int64)
        idxB = np.zeros(LB, np.int64)
        slocA = np.full(LA, -1.0, np.float32)
        slocB = np.full(LB, -1.0, np.float32)

        # slot (j) of each dst in its window
        slot_of = np.full(npc, -1, np.int64)
        nid = newid[c * npc : (c + 1) * npc]
        slot_of = (nid - c * NPS) % P
        win_dst = win_of

        for stream, msk, idx_arr, sloc_arr, Tcap, base in (
            ("A", a_c, idxA, slocA, TA, 0),
            ("B", ~a_c, idxB, slocB, TB, A_CORES * NPS),
        ):
            ss = s_c[msk]
            dd = d_c[msk]
            ww = win_dst[dd]
            jj = slot_of[dd]
            order = np.argsort(ww, kind="stable")
            ss, ww, jj = ss[order], ww[order], jj[order]
            cnt = np.bincount(ww, minlength=W)
            assert cnt.max(initial=0) <= Tcap * P
            starts = np.zeros(W + 1, np.int64)
            np.cumsum(cnt, out=starts[1:])
            pos = np.arange(len(ss)) - starts[ww] + ww * Tcap * P
            idx_arr[pos] = newid[ss] - base
            sloc_arr[pos] = jj.astype(np.float32)

        def wrap16(v):
            # token i -> [i % 16, i // 16], replicated 8x down partitions
            L = len(v)
            t = v.reshape(L // 16, 16).T.astype(np.int16).copy()
            return np.tile(t, (8, 1))

        h0_rows = slot_orig[c]  # [NPS] orig node or -1
        dinv_sl = np.where(h0_rows >= 0, dinv[np.maximum(h0_rows, 0)], 0.0)
        dinv_t = dinv_sl.reshape(W, P).T.astype(np.float32).copy()  # [128, W]

        # sloc buffer [128, W*T]: cols [0, W*TA) A-tiles, then B-tiles
        sloc_t = np.concatenate(
            [slocA.reshape(W * TA, P).T, slocB.reshape(W * TB, P).T], axis=1
        ).astype(np.float32).copy()

        per_core.append(
            dict(
                idxA=wrap16(idxA),
                idxB=wrap16(idxB),
                sloc=sloc_t,
                dinv=dinv_t,
                rows=h0_rows,
            )
        )

    meta = dict(N=N, E=E, npc=npc, W=W, TA=TA, TB=TB, T=T, NPS=NPS,
                LA=LA, LB=LB, A_ROWS=A_ROWS, B_ROWS=B_ROWS)
    return meta, per_core


# ----------------------------------------------------------------------------
# Device program
# ----------------------------------------------------------------------------

def build_program(meta, n_classes, hidden):
    W, TA, TB, T = meta["W"], meta["TA"], meta["TB"], meta["T"]
    NPS, LA, LB = meta["NPS"], meta["LA"], meta["LB"]
    A_ROWS, B_ROWS = meta["A_ROWS"], meta["B_ROWS"]
    TOT = M * NPS
    H = hidden
    C = n_classes

    nc = bacc.Bacc("TRN2", target_bir_lowering=False, debug=False, num_devices=M)

    h0_d = nc.dram_tensor("h0", [NPS, H], F32, kind="ExternalInput")
    dinv_d = nc.dram_tensor("dinv", [P, W], F32, kind="ExternalInput")
    sloc_d = nc.dram_tensor("sloc", [P, W * T], F32, kind="ExternalInput")
    idxA_d = nc.dram_tensor("idxA", [P, LA // 16], I16, kind="ExternalInput")
    idxB_d = nc.dram_tensor("idxB", [P, LB // 16], I16, kind="ExternalInput")
    w_d = [nc.dram_tensor(f"w{l}", [H, H], F32, kind="ExternalInput") for l in range(3)]
    b_d = [nc.dram_tensor(f"b{l}", [P, H], F32, kind="ExternalInput") for l in range(3)]
    wl_d = nc.dram_tensor("wl", [H, C], F32, kind="ExternalInput")
    bl_d = nc.dram_tensor("bl", [P, C], F32, kind="ExternalInput")
    iota_d = nc.dram_tensor("iota", [P, P], F32, kind="ExternalInput")
    ident_d = nc.dram_tensor("ident", [P, P], F32, kind="ExternalInput")
    out_d = nc.dram_tensor("out", [NPS, C], F32, kind="ExternalOutput")

    with tile.TileContext(nc) as tc:
        with (
            tc.tile_pool(name="const", bufs=1) as cpool,
            tc.tile_pool(name="hbuf", bufs=2) as hpool,
            tc.tile_pool(name="ubuf", bufs=2) as upool,
            tc.tile_pool(name="msgA", bufs=4) as mApool,
            tc.tile_pool(name="msgB", bufs=4) as mBpool,
            tc.tile_pool(name="idx", bufs=4) as ipool,
            tc.tile_pool(name="stile", bufs=4) as spool,
            tc.tile_pool(name="work", bufs=4) as wpool,
            tc.tile_pool(name="pacc", bufs=2, space="PSUM") as pacc,
            tc.tile_pool(name="ptr", bufs=2, space="PSUM") as ptr,
            tc.tile_pool(name="pz", bufs=2, space="PSUM") as pz,
            tc.tile_pool(name="dram", bufs=2, space="DRAM") as dpool,
            tc.tile_pool(name="dramu", bufs=2, space="DRAM") as dupool,
        ):
            # constants
            t_dinv = cpool.tile([P, W], F32)
            nc.sync.dma_start(out=t_dinv[:], in_=dinv_d[:])
            t_sloc = cpool.tile([P, W * T], F32)
            nc.sync.dma_start(out=t_sloc[:], in_=sloc_d[:])
            t_iota = cpool.tile([P, P], F32)
            nc.sync.dma_start(out=t_iota[:], in_=iota_d[:])
            t_ident = cpool.tile([P, P], F32)
            nc.sync.dma_start(out=t_ident[:], in_=ident_d[:])
            t_w = []
            t_b = []
            for l in range(3):
                tw = cpool.tile([H, H], F32, tag="wmat")
                nc.sync.dma_start(out=tw[:], in_=w_d[l][:])
                t_w.append(tw)
                tb = cpool.tile([P, H], F32, tag="bmat")
                nc.sync.dma_start(out=tb[:], in_=b_d[l][:])
                t_b.append(tb)
            t_wl = cpool.tile([H, C], F32)
            nc.sync.dma_start(out=t_wl[:], in_=wl_d[:])
            t_bl = cpool.tile([P, C], F32)
            nc.sync.dma_start(out=t_bl[:], in_=bl_d[:])

            # current h slice [128, W, H]
            t_h = hpool.tile([P, W, H], F32, tag="h")
            nc.sync.dma_start(
                out=t_h[:], in_=h0_d.rearrange("(w p) f -> p w f", p=P)
            )

            def compute_u(h_tile, w_tile):
                """u = dinv * (h @ W)  -> SBUF [128, W, H]"""
                u_sl = upool.tile([P, W, H], F32, tag="u")
                for w in range(W):
                    p_t = ptr.tile([P, H], F32, tag="ptr")
                    nc.tensor.transpose(out=p_t[:], in_=h_tile[:, w, :], identity=t_ident[:])
                    ht = wpool.tile([P, H], F32, tag="ht")
                    nc.vector.tensor_copy(out=ht[:], in_=p_t[:])
                    p_z = pz.tile([P, H], F32, tag="pz")
                    nc.tensor.matmul(p_z[:], lhsT=ht[:], rhs=w_tile[:], start=True, stop=True)
                    nc.vector.tensor_scalar_mul(u_sl[:, w, :], p_z[:], t_dinv[:, w : w + 1])
                return u_sl

            for l in range(3):
                u_sl = compute_u(t_h, t_w[l])

                # AllGather the u table
                ag_in = dpool.tile([NPS, H], F32, tag="agin")
                nc.sync.dma_start(
                    out=ag_in[:].rearrange("(w p) f -> p w f", p=P), in_=u_sl[:]
                )
                u_full = dupool.tile([TOT, H], F32, tag="ufull", addr_space="Shared")
                if USE_COLLECTIVE:
                    nc.gpsimd.collective_compute(
                        "AllGather",
                        mybir.AluOpType.bypass,
                        replica_groups=[list(range(M))],
                        ins=[ag_in.opt()],
                        outs=[u_full.opt()],
                    )
                else:
                    # debug: copy own slice into every core-slot position (SBUF src)
                    for c in range(M):
                        nc.sync.dma_start(
                            out=u_full[c * NPS : (c + 1) * NPS, :].rearrange(
                                "(w p) f -> p w f", p=P
                            ),
                            in_=u_sl[:],
                        )

                # rolling gather chunks per stream
                state = {}
                for sname, n_tiles, idx_d, pool, base_rows in (
                    ("A", W * TA, idxA_d, mApool, (0, A_ROWS)),
                    ("B", W * TB, idxB_d, mBpool, (A_ROWS, A_ROWS + B_ROWS)),
                ):
                    state[sname] = dict(n_tiles=n_tiles, idx_d=idx_d, pool=pool,
                                        base=base_rows, chunk=-1, tile=None)

                def msg_tile(sname, i):
                    st = state[sname]
                    c = i // CH
                    if c != st["chunk"]:
                        st["chunk"] = c
                        c0 = c * CH
                        c1 = min(c0 + CH, st["n_tiles"])
                        ntile = c1 - c0
                        nidx = ntile * P
                        t_idx = ipool.tile([P, CH * 8], I16, tag=f"idx{sname}")
                        nc.sync.dma_start(
                            out=t_idx[:, : nidx // 16],
                            in_=st["idx_d"][:, c0 * 8 : c0 * 8 + nidx // 16],
                        )
                        t_msg = st["pool"].tile([P, CH, H], F32, tag=f"msg{sname}")
                        r0, r1 = st["base"]
                        if USE_GATHER:
                            nc.gpsimd.dma_gather(
                                t_msg[:, :ntile, :],
                                u_full[r0:r1, :],
                                t_idx[:, : nidx // 16],
                                nidx,
                                nidx,
                                H,
                                single_packet=False,
                            )
                        st["tile"] = t_msg
                    return st["tile"][:, i % CH, :]

                h_next = hpool.tile([P, W, H], F32, tag="h")
                if not USE_MM:
                    for w in range(W):
                        for sname, Tn in (("A", TA), ("B", TB)):
                            for i in range(Tn):
                                msg_tile(sname, w * Tn + i)  # keep gather traffic
                        nc.vector.tensor_copy(h_next[:, w, :], u_sl[:, w, :])
                    t_h = h_next
                    continue
                for w in range(W):
                    p_acc = pacc.tile([P, H], F32, tag="pacc")
                    # self term: I @ u_w
                    nc.tensor.matmul(
                        p_acc[:], lhsT=t_ident[:], rhs=u_sl[:, w, :],
                        start=True, stop=False,
                    )
                    n_mm = TA + TB
                    k = 0
                    for sname, Tn, col0 in (("A", TA, w * TA), ("B", TB, W * TA + w * TB)):
                        for i in range(Tn):
                            col = col0 + i
                            s_t = spool.tile([P, P], F32, tag="s")
                            nc.vector.tensor_tensor(
                                out=s_t[:],
                                in0=t_sloc[:, col : col + 1].to_broadcast([P, P]),
                                in1=t_iota[:],
                                op=mybir.AluOpType.is_equal,
                            )
                            rhs = msg_tile(sname, w * Tn + i)
                            k += 1
                            nc.tensor.matmul(
                                p_acc[:], lhsT=s_t[:], rhs=rhs,
                                start=False, stop=(k == n_mm),
                            )
                    # epilogue: h = elu(dinv * acc + b)
                    y = wpool.tile([P, H], F32, tag="y")
                    nc.vector.tensor_scalar_mul(y[:], p_acc[:], t_dinv[:, w : w + 1])
                    nc.vector.tensor_add(y[:], y[:], t_b[l][:])
                    neg = wpool.tile([P, H], F32, tag="neg")
                    nc.vector.tensor_scalar_min(neg[:], y[:], 0.0)
                    e = wpool.tile([P, H], F32, tag="e")
                    nc.scalar.activation(e[:], neg[:], mybir.ActivationFunctionType.Exp)
                    pos = wpool.tile([P, H], F32, tag="pos")
                    nc.scalar.activation(pos[:], y[:], mybir.ActivationFunctionType.Relu)
                    nc.vector.tensor_add(h_next[:, w, :], pos[:], e[:])
                    nc.vector.tensor_scalar_add(h_next[:, w, :], h_next[:, w, :], -1.0)
                t_h = h_next

            # final linear + log_softmax
            for w in range(W):
                p_t = ptr.tile([P, H], F32, tag="ptr")
                nc.tensor.transpose(out=p_t[:], in_=t_h[:, w, :], identity=t_ident[:])
                ht = wpool.tile([P, H], F32, tag="ht")
                nc.vector.tensor_copy(out=ht[:], in_=p_t[:])
                p_lg = pz.tile([P, C], F32, tag="pz")
                nc.tensor.matmul(p_lg[:], lhsT=ht[:], rhs=t_wl[:], start=True, stop=True)
                lg = wpool.tile([P, C], F32, tag="lg")
                nc.vector.tensor_add(lg[:], p_lg[:], t_bl[:])
                negm = wpool.tile([P, 1], F32, tag="negm")
                nc.vector.tensor_reduce(
                    negm[:], lg[:], axis=mybir.AxisListType.X,
                    op=mybir.AluOpType.max, negate=True,
                )
                escr = wpool.tile([P, C], F32, tag="escr")
                ssum = wpool.tile([P, 1], F32, tag="ssum")
                nc.scalar.activation(
                    escr[:], lg[:], mybir.ActivationFunctionType.Exp,
                    bias=negm[:, :1], accum_out=ssum[:, :1],
                )
                lns = wpool.tile([P, 1], F32, tag="lns")
                nc.scalar.activation(lns[:], ssum[:], mybir.ActivationFunctionType.Ln)
                shift = wpool.tile([P, 1], F32, tag="shift")
                nc.vector.tensor_sub(shift[:], negm[:], lns[:])
                o_t = wpool.tile([P, C], F32, tag="ot")
                nc.vector.tensor_scalar_add(o_t[:], lg[:], shift[:, :1])
                nc.sync.dma_start(
                    out=out_d.rearrange("(w p) c -> p w c", p=P)[:, w, :], in_=o_t[:]
                )

    nc.compile()
    return nc


def Tn_of(sname, TA, TB):
    return TA if sname == "A" else TB


# ----------------------------------------------------------------------------
# Entry point
# ----------------------------------------------------------------------------

_CACHE = {}
LAST_EXEC_NS = None


def _prepare(x, edge_index, W0, b0, W1, b1, W2, b2, Wl, bl):
    x = np.asarray(x)
    edge_index = np.asarray(edge_index)
    N, H = x.shape
    C = np.asarray(Wl).shape[1]

    meta, per_core = build_schedule(edge_index, N)
    key = (N, edge_index.shape[1], H, C, meta["W"], meta["TA"], meta["TB"])
    if key not in _CACHE:
        _CACHE[key] = build_program(meta, C, H)
    nc = _CACHE[key]

    NPS, W = meta["NPS"], meta["W"]
    iota = np.tile(np.arange(P, dtype=np.float32), (P, 1))
    ident = np.eye(P, dtype=np.float32)
    bb = [np.tile(np.asarray(b, np.float32), (P, 1)) for b in (b0, b1, b2)]
    blb = np.tile(np.asarray(bl, np.float32), (P, 1))

    in_maps = []
    for c in range(M):
        pc = per_core[c]
        rows = pc["rows"]
        h0 = np.zeros((NPS, H), np.float32)
        valid = rows >= 0
        h0[valid] = np.asarray(x, np.float32)[rows[valid]]
        in_maps.append(
            dict(
                h0=h0,
                dinv=pc["dinv"],
                sloc=pc["sloc"],
                idxA=pc["idxA"],
                idxB=pc["idxB"],
                w0=np.asarray(W0, np.float32),
                w1=np.asarray(W1, np.float32),
                w2=np.asarray(W2, np.float32),
                b0=bb[0], b1=bb[1], b2=bb[2],
                wl=np.asarray(Wl, np.float32),
                bl=blb,
                iota=iota,
                ident=ident,
            )
        )

    return nc, in_maps, meta, per_core, (N, C)


def _assemble(res, per_core, N, C):
    out = np.zeros((N, C), np.float32)
    for c in range(M):
        rows = per_core[c]["rows"]
        valid = rows >= 0
        out[rows[valid]] = res.results[c]["out"][valid]
    return out


def kernel(x, edge_index, W0, b0, W1, b1, W2, b2, Wl, bl):
    global LAST_EXEC_NS
    nc, in_maps, meta, per_core, (N, C) = _prepare(
        x, edge_index, W0, b0, W1, b1, W2, b2, Wl, bl
    )
    res = run_bass_kernel_spmd(nc, in_maps, list(range(M)))
    LAST_EXEC_NS = res.exec_time_ns
    return _assemble(res, per_core, N, C)


def profile_once(inputs):
    nc, in_maps, meta, per_core, (N, C) = _prepare(**inputs)
    res = run_bass_kernel_spmd(nc, in_maps, list(range(M)), trace=True)
    return res.exec_time_ns

